# revision 13
# baseline (speedup 1.0000x reference)
"""MoE grouped-GEMM (8 experts) on 8 Trainium2 NeuronCores.

Problem: input [32768, 1024] routed contiguously to 8 experts (counts in
num_experts_per_token); expert i computes x_i @ W_i.T + b_i with
W [8, 4096, 1024], b [8, 4096]. Output [32768, 4096].

Sharding: expert-parallel, expert i <-> core i. Zero collectives: the host
slices each expert's token block, packs x and W into SBUF tile layout
(contraction dim DIN on partitions) in bf16, each core runs a
4096x1024x4096 GEMM (+bias), and the host concatenates per-core outputs.

Why bf16 (measured on this silicon, micro-benched):
  - f32r MM N=512 sustains 227 ns/inst: the f32r LDWEIGHTS (187 ns, no
    FWL) plus handoff sets the pace, not the 213 ns stream.
  - bf16 MM N=512 sustains 216 ns/inst (= 512 cyc @ 2.4 GHz + ~3 cyc NX):
    FWL drops LDWEIGHTS to 97 ns, fully hidden by the PE reorder window.
  - fp8 DoubleRow also paces at 216 ns/inst (2x MACs/inst) but raw fp8
    accuracy (~5%) fails and error-corrected fp8 needs 1.5x instructions.
  - matmul PSUM output cannot span banks -> N=512 max, 2048 MMs/core.
  bf16 steady-state floor: 2048 x 216 ns = 442 us. bf16 input
  quantization rel err ~1.6e-3 (gate is 2e-2).

Schedule (per core): x (8 MB) and W (8 MB) are fully SBUF-resident,
loaded once (no restreaming). Warm-up matmuls on a memset tile run while
the first DMAs land so the PE HAM clock-gate (1.2 -> 2.4 GHz after
~3.4 us busy) is warm when real work starts; first real MMs gate on
k-sliced w[n0] + x[m0] only. 8 PSUM banks rotate k-accumulation groups;
DVE fuses bias-add with PSUM drain; outputs stream on the scalar ring.
"""

import sys

if "/opt/trn_rl_repo" not in sys.path:
    sys.path.insert(0, "/opt/trn_rl_repo")

import numpy as np
import ml_dtypes

E, T, DIN, DOUT = 8, 32768, 1024, 4096
NCORES = 8
TOKC = T // NCORES  # tokens per core (capacity)

KT = 128   # contraction tile (SBUF partitions)
MT = 128   # token tile (PSUM partitions)
NT = 512   # dout tile (one fp32 PSUM bank)
KTILES = DIN // KT    # 8
MTILES = TOKC // MT   # 32
NTILES = DOUT // NT   # 8

WARMUP_MMS = 7

_CACHE = {}


def _build_nc():
    import concourse.bacc as bacc
    import concourse.tile as tile
    import concourse.mybir as mybir

    nc = bacc.Bacc("TRN2", target_bir_lowering=False, debug=False,
                   num_devices=NCORES)

    bf16 = mybir.dt.bfloat16
    xP = nc.dram_tensor("xP", [KT, MTILES, KTILES * MT], bf16,
                        kind="ExternalInput")
    wP = nc.dram_tensor("wP", [KT, NTILES, KTILES * NT], bf16,
                        kind="ExternalInput")
    bias_b = nc.dram_tensor("bias_b", [MT, DOUT], mybir.dt.float32,
                            kind="ExternalInput")
    y = nc.dram_tensor("y", [TOKC, DOUT], mybir.dt.float32,
                       kind="ExternalOutput")

    with tile.TileContext(nc) as tc:
        with (
            tc.tile_pool(name="sb", bufs=1) as sb,
            tc.tile_pool(name="opool", bufs=8) as opool,
            tc.tile_pool(name="psum", bufs=8, space="PSUM") as psum_pool,
        ):
            # HAM warm-up: memset a small bf16 tile, run throwaway MMs on
            # it so the PE is at 2.4 GHz by the time real data lands.
            wu = sb.tile([KT, NT], bf16, name="wu", tag="wu")
            nc.vector.memset(wu[:], 1.0)
            for i in range(WARMUP_MMS):
                wacc = psum_pool.tile([MT, NT], mybir.dt.float32,
                                      name="acc", tag="acc")
                nc.tensor.matmul(wacc[:], wu[:, 0:MT], wu[:],
                                 start=True, stop=True)

            xt = sb.tile([KT, MTILES, KTILES * MT], bf16,
                         name="xt", tag="xt")
            wt = sb.tile([KT, NTILES, KTILES * NT], bf16,
                         name="wt", tag="wt")
            bias_t = sb.tile([MT, DOUT], mybir.dt.float32,
                             name="bias_t", tag="bias_t")

            # input DMAs, all on the sync ring, in consumption order: the
            # first MM gates on w[n0,k0] (128 KB) + x[m0] (256 KB) only;
            # bias arrives n-sliced (first drain needs just slice 0).
            # NOTE: rows < ~1 KB make DMAs descriptor-bound (measured: 32 KB
            # [128x256B] x-slices took ~1.3 us each, 4x slower per byte), so
            # the x[m0] gate stays one [128x2KB] transfer.
            nc.sync.dma_start(wt[:, 0, 0:NT], wP[:, 0, 0:NT])
            # x[m0], bias slice 0, and x[m1] ride the scalar queue (idle
            # until the first drain) so gating transfers land in parallel.
            nc.scalar.dma_start(xt[:, 0], xP[:, 0])
            nc.scalar.dma_start(bias_t[:, 0:NT], bias_b[:, 0:NT])
            nc.scalar.dma_start(xt[:, 1], xP[:, 1])
            # rest of w[n0] coalesced into 4KB/3KB-row transfers (full DMA
            # efficiency) -- the k0 gate above stays small for earliest start
            nc.sync.dma_start(wt[:, 0, NT:5 * NT], wP[:, 0, NT:5 * NT])
            nc.sync.dma_start(wt[:, 0, 5 * NT:], wP[:, 0, 5 * NT:])
            for m in range(2, MTILES):
                nc.sync.dma_start(xt[:, m], xP[:, m])
            nc.sync.dma_start(bias_t[:, NT:], bias_b[:, NT:])
            for n in range(1, NTILES):
                nc.sync.dma_start(wt[:, n], wP[:, n])

            for n in range(NTILES):
                for m in range(MTILES):
                    acc = psum_pool.tile([MT, NT], mybir.dt.float32,
                                         name="acc", tag="acc")
                    for k in range(KTILES):
                        nc.tensor.matmul(
                            acc[:],
                            xt[:, m, k * MT:(k + 1) * MT],
                            wt[:, n, k * NT:(k + 1) * NT],
                            start=(k == 0), stop=(k == KTILES - 1))
                    ot = opool.tile([MT, NT], mybir.dt.float32,
                                    name="ot", tag="ot")
                    nc.vector.tensor_add(
                        ot[:], acc[:], bias_t[:, n * NT:(n + 1) * NT])
                    nc.scalar.dma_start(
                        y[m * MT:(m + 1) * MT, n * NT:(n + 1) * NT], ot[:])

    nc.compile()
    return nc


def _install_neff_cache():
    """Disk-cache walrus NEFF compiles keyed on the BIR bytes."""
    if _CACHE.get("neff_cache_installed"):
        return
    _CACHE["neff_cache_installed"] = True
    import hashlib
    import os
    import shutil

    import concourse.bass2jax as bass2jax

    cache_dir = "/root/.neff_bir_cache"
    os.makedirs(cache_dir, exist_ok=True)
    orig = bass2jax.compile_bir_kernel

    def cached_compile(ant_bir_str, tmpdir, neff_name="file.neff", **kw):
        key = hashlib.sha256(
            ant_bir_str if isinstance(ant_bir_str, bytes)
            else ant_bir_str.encode()).hexdigest()
        hit = os.path.join(cache_dir, key + ".neff")
        dst = os.path.join(tmpdir, neff_name)
        if os.path.exists(hit):
            shutil.copyfile(hit, dst)
            return dst
        out = orig(ant_bir_str, tmpdir, neff_name=neff_name, **kw)
        try:
            shutil.copyfile(out, hit)
        except OSError:
            pass
        return out

    bass2jax.compile_bir_kernel = cached_compile


def _get_nc():
    if "nc" not in _CACHE:
        _install_neff_cache()
        _CACHE["nc"] = _build_nc()
    return _CACHE["nc"]


def _pack_x(xi):
    """[TOKC, DIN] f32 -> [128, MTILES, KTILES*MT] bf16 where
    out[p, m, k*MT + t] = xi[m*MT + t, k*KT + p]."""
    return np.ascontiguousarray(
        xi.reshape(MTILES, MT, KTILES, KT).transpose(3, 0, 2, 1)
        .reshape(KT, MTILES, KTILES * MT)).astype(ml_dtypes.bfloat16)


def _pack_w(wi):
    """[DOUT, DIN] f32 -> [128, NTILES, KTILES*NT] bf16 where
    out[p, n, k*NT + d] = wi[n*NT + d, k*KT + p]."""
    return np.ascontiguousarray(
        wi.reshape(NTILES, NT, KTILES, KT).transpose(3, 0, 2, 1)
        .reshape(KT, NTILES, KTILES * NT)).astype(ml_dtypes.bfloat16)


def kernel(input, weight, bias, num_experts_per_token):
    from concourse.bass_utils import run_bass_kernel_spmd

    input = np.ascontiguousarray(np.asarray(input, dtype=np.float32))
    weight = np.ascontiguousarray(np.asarray(weight, dtype=np.float32))
    bias = np.ascontiguousarray(np.asarray(bias, dtype=np.float32))
    counts = np.asarray(num_experts_per_token).astype(np.int64)
    offsets = np.concatenate([[0], np.cumsum(counts)]).astype(np.int64)

    if counts.max() > TOKC:
        # capacity overflow (never hit with balanced routing): numpy fallback
        outs = []
        for i in range(E):
            xi = input[offsets[i]:offsets[i + 1]]
            outs.append(xi @ weight[i].T + bias[i])
        return np.concatenate(outs, axis=0)

    in_maps = []
    for i in range(E):
        xi = input[offsets[i]:offsets[i + 1]]  # [n_i, DIN]
        if xi.shape[0] < TOKC:
            xi = np.concatenate(
                [xi, np.zeros((TOKC - xi.shape[0], DIN), np.float32)], axis=0)
        bb = np.ascontiguousarray(
            np.broadcast_to(bias[i][None, :], (MT, DOUT)))
        in_maps.append({"xP": _pack_x(xi), "wP": _pack_w(weight[i]),
                        "bias_b": bb})

    nc = _get_nc()
    import os
    trace = bool(int(os.environ.get("KERNEL_TRACE", "0")))
    res = run_bass_kernel_spmd(nc, in_maps, core_ids=list(range(NCORES)),
                               trace=trace)
    _CACHE["last_result"] = res

    out = np.empty((T, DOUT), dtype=np.float32)
    pos = 0
    for i in range(E):
        n_i = int(counts[i])
        out[pos:pos + n_i] = res.results[i]["y"][:n_i]
        pos += n_i
    return out
